# revision 11
# baseline (speedup 1.0000x reference)
"""Trainium2 Bass kernel for nn_DetectorKe_652835029279 (Gaussian-mixture
log-likelihood detector: weighted logsumexp over 256 Mahalanobis distances).

Math: ll_i = log sum_j coef_j * exp(-0.5 * (x_i-c_j)^T A_j (x_i-c_j)) - thr
    = logsumexp_j( -0.5 * x^T A_j x + x . (A_j c_j) + bias_j )
with bias_j = log(coef_j) - 0.5 c_j^T A_j c_j - thr folded in, and the
quadratic term expanded over 17 cyclic-rotation pair blocks
(d, (d+k) % 32), k = 0..16 (544 pair slots, off-diagonal coefficients
doubled), so each row reduces to ONE K-dim contraction
  d'[i, j] = sum_s G[i, s] * U[s, j]
with G built on-chip and U precomputed on host.

K is packed to 5 chunks (640): chunks 0..3 are full 128-row pair-product
blocks (k = 4c + p//32) in bf16 (xx products and U coefficients rounded
to bf16 - rel err ~1.4e-3, 14x under the 2e-2 gate); chunk 4 is a K=65
f32r chunk [xx_k16 (32); X^T (32); ones (1)] carrying the k=16 block,
the linear term and the bias at full precision.

Device layout per core (data-parallel over N, 16384 rows/core), per
512-row tile: DMA X -> 4 PE transposes -> X^T [32,512] -> K=32 selection
matmuls build the rotated copies (no zero-padding streamed) -> DVE
multiplies build pair products (bf16 out) -> 20 accumulating matmuls
into PSUM [128,1024] -> ACT exp with fused free-dim accumulate ->
single Ln + PE transpose + DMA straight from PSUM at the end.
"""
import sys

if "/opt/trn_rl_repo" not in sys.path:
    sys.path.insert(0, "/opt/trn_rl_repo")

import numpy as np

N, D, M = 131072, 32, 256
NCORES = 8
NC_ROWS = N // NCORES          # 16384
TILE_ROWS = 512
NTILES = NC_ROWS // TILE_ROWS  # 32
NGROUPS = NC_ROWS // 128       # 128

_PROGRAM = None


def _build_program():
    import concourse.bacc as bacc
    import concourse.mybir as mybir
    import concourse.tile as tile

    f32 = mybir.dt.float32
    f32r = mybir.dt.float32r
    bf16 = mybir.dt.bfloat16
    AF = mybir.ActivationFunctionType

    nc = bacc.Bacc(None, target_bir_lowering=False)
    X_d = nc.dram_tensor("X", [NC_ROWS, D], f32r, kind="ExternalInput")
    U_d = nc.dram_tensor("U", [128, 4, M], bf16, kind="ExternalInput")
    U4_d = nc.dram_tensor("U4", [65, M], f32r, kind="ExternalInput")
    SEL_d = nc.dram_tensor("SEL", [32, 704], f32r, kind="ExternalInput")
    ONES_d = nc.dram_tensor("ONES", [1, TILE_ROWS], f32r, kind="ExternalInput")
    EYER_d = nc.dram_tensor("EYER", [128, 128], f32r, kind="ExternalInput")
    OUT_d = nc.dram_tensor("out", [NC_ROWS], f32, kind="ExternalOutput")

    with tile.TileContext(nc) as tc:
        with (
            tc.tile_pool(name="const", bufs=1) as constp,
            tc.tile_pool(name="xin", bufs=3) as xinp,
            tc.tile_pool(name="xtp", bufs=2) as xtpool,
            tc.tile_pool(name="c4p", bufs=2) as c4pool,
            tc.tile_pool(name="xt4p", bufs=2) as xt4pool,
            tc.tile_pool(name="xxp", bufs=2) as xxpool,
            tc.tile_pool(name="expp", bufs=4) as exppool,
            tc.tile_pool(name="sumsp", bufs=1) as sumspool,
            tc.tile_pool(name="finp", bufs=1) as finpool,
            tc.tile_pool(name="ps_xt", bufs=2, space="PSUM") as ps_xt,
            tc.tile_pool(name="ps_xt4", bufs=1, space="PSUM") as ps_xt4,
            tc.tile_pool(name="ps_rot", bufs=2, space="PSUM") as ps_rot,
            tc.tile_pool(name="ps_main", bufs=3, space="PSUM") as ps_main,
        ):
            # startup order matters: EYER + SEL + X0 gate the first tile;
            # U/U4 aren't needed until the first main matmul, so their
            # dma_start dispatches are deferred into the loop body.
            EYER_sb = constp.tile([128, 128], f32r)
            nc.sync.dma_start(EYER_sb[:], EYER_d[:])
            SEL_sb = constp.tile([32, 704], f32r)
            nc.sync.dma_start(SEL_sb[:], SEL_d[:])
            U_sb = constp.tile([128, 4, M], bf16)
            U4_sb = constp.tile([65, M], f32r)

            sums_sb = sumspool.tile([128, NGROUPS], f32)

            # persistent double-buffered tiles: xt = X^T [32,512];
            # c4 = [xx16 (0:32); X^T (32:64); ones (64)], row 64 set once.
            xt_tiles = []
            c4_tiles = []
            for i in range(2):
                xt_p = xtpool.tile(
                    [32, TILE_ROWS], f32r, tag=f"xtP{i}", bufs=1, name=f"xt_p{i}"
                )
                xt_tiles.append(xt_p)
                c4_p = c4pool.tile(
                    [65, TILE_ROWS], f32r, tag=f"c4P{i}", bufs=1, name=f"c4_p{i}"
                )
                nc.sync.dma_start(c4_p[64:65, :], ONES_d[:])
                c4_tiles.append(c4_p)

            for t in range(NTILES):
                x_t = xinp.tile([128, 4 * D], f32r, tag="x")
                nc.sync.dma_start(
                    x_t[:].rearrange("p (g d) -> p g d", g=4),
                    X_d[t * TILE_ROWS : (t + 1) * TILE_ROWS, :].rearrange(
                        "(g p) d -> p g d", p=128
                    ),
                )
                if t == 0:
                    nc.sync.dma_start(U_sb[:], U_d[:])
                    nc.sync.dma_start(U4_sb[:], U4_d[:])

                # X^T [32, 512] via 4 PE transposes
                xtps = ps_xt.tile([32, TILE_ROWS], f32r, tag="xtps")
                for g in range(4):
                    nc.tensor.transpose(
                        xtps[:, g * 128 : (g + 1) * 128],
                        x_t[:, g * D : (g + 1) * D],
                        EYER_sb[:],
                    )
                xt_sb = xt_tiles[t % 2]
                nc.scalar.copy(xt_sb[:], xtps[:])

                c4_sb = c4_tiles[t % 2]
                # X^T into c4 rows 32:64 (SBUF->SBUF on the idle gpsimd)
                nc.gpsimd.tensor_copy(c4_sb[32:64, :], xt_sb[:])

                # rot16 first (feeds the earliest main matmul of each group)
                rotps4 = ps_rot.tile([32, TILE_ROWS], f32, tag="rot")
                nc.tensor.matmul(
                    rotps4[:], SEL_sb[:, 640:672], xt_sb[:], start=True, stop=True
                )

                # XT4 = 4-fold stack of X^T (partition p holds x_{p%32})
                xt4ps = ps_xt4.tile([128, TILE_ROWS], f32, tag="xt4ps")
                nc.tensor.matmul(
                    xt4ps[:], SEL_sb[:, 0:128], xt_sb[:], start=True, stop=True
                )
                xt4_sb = xt4pool.tile([128, TILE_ROWS], f32r, tag="xt4")
                nc.scalar.copy(xt4_sb[:], xt4ps[:])

                # xx16 into c4 rows 0:32
                nc.vector.tensor_mul(c4_sb[0:32, :], xt4_sb[0:32, :], rotps4[:])

                # pair-product chunks 0..3 (bf16):
                #   chunk_c[p] = x_{p%32} * x_{(p%32 + 4c + p//32)%32}
                chunk_tiles = []
                for c in range(4):
                    rotps = ps_rot.tile([128, TILE_ROWS], f32, tag="rot")
                    nc.tensor.matmul(
                        rotps[:],
                        SEL_sb[:, 128 * (c + 1) : 128 * (c + 2)],
                        xt_sb[:],
                        start=True,
                        stop=True,
                    )
                    xx_c = xxpool.tile([128, TILE_ROWS], bf16, tag=f"xx{c}")
                    nc.vector.tensor_mul(xx_c[:], xt4_sb[:], rotps[:])
                    chunk_tiles.append(xx_c)

                # main accumulating matmuls; c4 (f32r, K=65) opens each
                # group, the four bf16 K=128 chunks follow.
                for half in range(2):
                    psmain = ps_main.tile([128, 2 * M], f32, tag="main")
                    for s2 in range(2):
                        sub = half * 2 + s2
                        nc.tensor.matmul(
                            psmain[:, s2 * M : (s2 + 1) * M],
                            c4_sb[:, sub * 128 : (sub + 1) * 128],
                            U4_sb[:],
                            start=True,
                            stop=False,
                        )
                        for c in range(4):
                            nc.tensor.matmul(
                                psmain[:, s2 * M : (s2 + 1) * M],
                                chunk_tiles[c][:, sub * 128 : (sub + 1) * 128],
                                U_sb[:, c, :],
                                start=False,
                                stop=(c == 3),
                            )
                    for s2 in range(2):
                        sub = half * 2 + s2
                        expsc = exppool.tile([128, M], bf16, tag="exp")
                        col = t * 4 + sub
                        nc.scalar.activation(
                            expsc[:],
                            psmain[:, s2 * M : (s2 + 1) * M],
                            AF.Exp,
                            accum_out=sums_sb[:, col : col + 1],
                        )

            # epilogue: ll^T = Ln(sums); PE transpose; DMA straight from PSUM
            llT = finpool.tile([128, NGROUPS], f32r)
            nc.scalar.activation(llT[:], sums_sb[:], AF.Ln)
            llps = ps_xt.tile([128, 128], f32r, tag="xtps")
            nc.tensor.transpose(llps[:], llT[:], EYER_sb[:])
            ll_sb = finpool.tile([128, 128], f32)
            nc.scalar.copy(ll_sb[:], llps[:])
            nc.sync.dma_start(OUT_d.rearrange("(c p) -> c p", c=128), ll_sb[:])

    nc.compile()
    return nc


def _host_prep(center, cov_inv_sqrt, weight, threshold):
    import ml_dtypes

    L = np.asarray(cov_inv_sqrt, dtype=np.float64)
    w = np.abs(np.asarray(weight, dtype=np.float64))
    pr = w / w.sum()
    A = np.einsum("mij,mkj->mik", L, L)
    sign, logdet = np.linalg.slogdet(A)
    logcoef = np.log(pr) + 0.5 * logdet
    c64 = np.asarray(center, dtype=np.float64)
    Ac = np.einsum("mkl,ml->mk", A, c64)
    term3 = np.einsum("mk,mk->m", c64, Ac)
    bias = logcoef - 0.5 * term3 - float(np.asarray(threshold).reshape(-1)[0])

    p = np.arange(128)
    U = np.zeros((128, 4, M), np.float32)
    for c in range(4):
        k = 4 * c + p // 32
        d1 = p % 32
        d2 = (d1 + k) % 32
        mult = np.where(k == 0, 1.0, 2.0)
        U[:, c, :] = (-0.5 * mult[:, None] * A[:, d1, d2].T).astype(np.float32)

    p32 = np.arange(32)
    U4 = np.zeros((65, M), np.float32)
    U4[0:32] = (-0.5 * A[:, p32, (p32 + 16) % 32].T).astype(np.float32)
    U4[32:64] = Ac.T.astype(np.float32)
    U4[64] = bias.astype(np.float32)

    SEL = np.zeros((32, 704), np.float32)
    dd = np.arange(32)
    SEL[:, 0:128] = (dd[:, None] == (p[None, :] % 32)).astype(np.float32)
    for c in range(4):
        k = 4 * c + p // 32
        b = (p % 32 + k) % 32
        SEL[:, 128 * (c + 1) : 128 * (c + 2)] = (dd[:, None] == b[None, :]).astype(
            np.float32
        )
    b16 = (p32 + 16) % 32
    SEL[:, 640:672] = (dd[:, None] == b16[None, :]).astype(np.float32)

    EYER = np.eye(128, dtype=np.float32)
    ONES = np.ones((1, TILE_ROWS), np.float32)
    return U.astype(ml_dtypes.bfloat16), U4, SEL, EYER, ONES


def kernel(X, center, cov_inv_sqrt, weight, threshold):
    global _PROGRAM
    from concourse.bass_utils import run_bass_kernel_spmd

    X = np.ascontiguousarray(np.asarray(X, dtype=np.float32))
    U, U4, SEL, EYER, ONES = _host_prep(center, cov_inv_sqrt, weight, threshold)

    if _PROGRAM is None:
        _PROGRAM = _build_program()
    nc = _PROGRAM

    in_maps = []
    for k in range(NCORES):
        in_maps.append(
            {
                "X": X[k * NC_ROWS : (k + 1) * NC_ROWS],
                "U": U,
                "U4": U4,
                "SEL": SEL,
                "EYER": EYER,
                "ONES": ONES,
            }
        )
    res = run_bass_kernel_spmd(nc, in_maps, list(range(NCORES)))
    out = np.concatenate([res.results[k]["out"] for k in range(NCORES)])
    return out.astype(np.float32)


# revision 12
# speedup vs baseline: 1.6463x; 1.6463x over previous
"""Trainium2 Bass kernel for nn_DetectorKe_652835029279 (Gaussian-mixture
log-likelihood detector: weighted logsumexp over 256 Mahalanobis distances).

Math: ll_i = log sum_j coef_j * exp(-0.5 * (x_i-c_j)^T A_j (x_i-c_j)) - thr
    = logsumexp_j( -0.5 * x^T A_j x + x . (A_j c_j) + bias_j )
with bias_j = log(coef_j) - 0.5 c_j^T A_j c_j - thr folded in, and the
quadratic term expanded over 17 cyclic-rotation pair blocks
(d, (d+k) % 32), k = 0..16 (544 pair slots, off-diagonal coefficients
doubled), so each row reduces to ONE K-dim contraction
  d'[i, j] = sum_s G[i, s] * U[s, j]
with G built on-chip and U precomputed on host.

K is packed to 5 chunks of 128 (vs 6 in the original layout): chunks
0..3 are full pair-product blocks (k = 4c + p//32) in bf16 (xx products
and U coefficients rounded to bf16 - rel err ~1.5e-3, 13x under the
2e-2 gate); chunk 4 is f32r [xx_k16 (32); X^T (32); ones (1); zeros]
carrying the k=16 block, the linear term and the bias at full precision.

ALL matmuls keep K=128 (pad rows exact zeros on both operands): K<128
matmuls stream at half rate on trn2 AND starve the PE HAM activity
monitor so the clock never un-throttles from 1.2 GHz (measured: a K=32
variant ran the whole kernel cold, 313us vs 196us).

Device layout per core (data-parallel over N, 16384 rows/core), per
512-row tile: DMA X -> 4 PE transposes -> X^T [32,512] -> 6 selection
matmuls build rotated copies -> DVE multiplies build pair products
(bf16 out) -> 20 accumulating matmuls into PSUM [128,1024] -> ACT exp
with fused free-dim accumulate -> Ln + PE transpose + DMA at the end.
"""
import sys

if "/opt/trn_rl_repo" not in sys.path:
    sys.path.insert(0, "/opt/trn_rl_repo")

import numpy as np

N, D, M = 131072, 32, 256
NCORES = 8
NC_ROWS = N // NCORES          # 16384
TILE_ROWS = 512
NTILES = NC_ROWS // TILE_ROWS  # 32
NGROUPS = NC_ROWS // 128       # 128

_PROGRAM = None


def _build_program():
    import concourse.bacc as bacc
    import concourse.mybir as mybir
    import concourse.tile as tile

    f32 = mybir.dt.float32
    f32r = mybir.dt.float32r
    bf16 = mybir.dt.bfloat16
    AF = mybir.ActivationFunctionType

    nc = bacc.Bacc(None, target_bir_lowering=False)
    X_d = nc.dram_tensor("X", [NC_ROWS, D], f32r, kind="ExternalInput")
    U_d = nc.dram_tensor("U", [128, 4, M], bf16, kind="ExternalInput")
    U4_d = nc.dram_tensor("U4", [128, M], f32r, kind="ExternalInput")
    SEL_d = nc.dram_tensor("SEL", [128, 704], f32r, kind="ExternalInput")
    EYER_d = nc.dram_tensor("EYER", [128, 128], f32r, kind="ExternalInput")
    # PAD4 fills c4 rows 64:128 = [ones; zeros] once; PADX fills xt rows
    # 32:128 = zeros once (sel matmuls contract over all 128 partitions).
    PAD4_d = nc.dram_tensor("PAD4", [64, TILE_ROWS], f32r, kind="ExternalInput")
    PADX_d = nc.dram_tensor("PADX", [96, TILE_ROWS], f32r, kind="ExternalInput")
    OUT_d = nc.dram_tensor("out", [NC_ROWS], f32, kind="ExternalOutput")

    with tile.TileContext(nc) as tc:
        with (
            tc.tile_pool(name="const", bufs=1) as constp,
            tc.tile_pool(name="xin", bufs=3) as xinp,
            tc.tile_pool(name="xtp", bufs=2) as xtpool,
            tc.tile_pool(name="c4p", bufs=2) as c4pool,
            tc.tile_pool(name="xt4p", bufs=2) as xt4pool,
            tc.tile_pool(name="xxp", bufs=2) as xxpool,
            tc.tile_pool(name="expp", bufs=4) as exppool,
            tc.tile_pool(name="sumsp", bufs=1) as sumspool,
            tc.tile_pool(name="finp", bufs=1) as finpool,
            tc.tile_pool(name="ps_xt", bufs=2, space="PSUM") as ps_xt,
            tc.tile_pool(name="ps_xt4", bufs=1, space="PSUM") as ps_xt4,
            tc.tile_pool(name="ps_rot", bufs=2, space="PSUM") as ps_rot,
            tc.tile_pool(name="ps_main", bufs=3, space="PSUM") as ps_main,
        ):
            # startup order matters: EYER + SEL + X0 gate the first tile;
            # U/U4 aren't needed until the first main matmul, so their
            # dma_start dispatches are deferred into the loop body.
            EYER_sb = constp.tile([128, 128], f32r)
            nc.sync.dma_start(EYER_sb[:], EYER_d[:])
            SEL_sb = constp.tile([128, 704], f32r)
            nc.sync.dma_start(SEL_sb[:], SEL_d[:])
            U_sb = constp.tile([128, 4, M], bf16)
            U4_sb = constp.tile([128, M], f32r)

            sums_sb = sumspool.tile([128, NGROUPS], f32)

            # persistent double-buffered tiles; per-tile writes only touch
            # the live rows, the DMA'd pad stays valid across reuse.
            xt_tiles = []
            c4_tiles = []
            for i in range(2):
                xt_p = xtpool.tile(
                    [128, TILE_ROWS], f32r, tag=f"xtP{i}", bufs=1, name=f"xt_p{i}"
                )
                nc.sync.dma_start(xt_p[32:128, :], PADX_d[:])
                xt_tiles.append(xt_p)
                c4_p = c4pool.tile(
                    [128, TILE_ROWS], f32r, tag=f"c4P{i}", bufs=1, name=f"c4_p{i}"
                )
                nc.sync.dma_start(c4_p[64:128, :], PAD4_d[:])
                c4_tiles.append(c4_p)

            for t in range(NTILES):
                x_t = xinp.tile([128, 4 * D], f32r, tag="x")
                nc.sync.dma_start(
                    x_t[:].rearrange("p (g d) -> p g d", g=4),
                    X_d[t * TILE_ROWS : (t + 1) * TILE_ROWS, :].rearrange(
                        "(g p) d -> p g d", p=128
                    ),
                )
                if t == 0:
                    nc.sync.dma_start(U_sb[:], U_d[:])
                    nc.sync.dma_start(U4_sb[:], U4_d[:])

                # X^T [32, 512] via 4 PE transposes
                xtps = ps_xt.tile([32, TILE_ROWS], f32r, tag="xtps")
                for g in range(4):
                    nc.tensor.transpose(
                        xtps[:, g * 128 : (g + 1) * 128],
                        x_t[:, g * D : (g + 1) * D],
                        EYER_sb[:],
                    )
                xt_sb = xt_tiles[t % 2]
                nc.scalar.copy(xt_sb[:32, :], xtps[:])

                c4_sb = c4_tiles[t % 2]
                # X^T into c4 rows 32:64 (SBUF->SBUF on the idle gpsimd)
                nc.gpsimd.tensor_copy(c4_sb[32:64, :], xt_sb[:32, :])

                # rot16 first (its DVE product gates c4)
                rotps4 = ps_rot.tile([32, TILE_ROWS], f32, tag="rot")
                nc.tensor.matmul(
                    rotps4[:], SEL_sb[:, 640:672], xt_sb[:], start=True, stop=True
                )

                # XT4 = 4-fold stack of X^T (partition p holds x_{p%32})
                xt4ps = ps_xt4.tile([128, TILE_ROWS], f32, tag="xt4ps")
                nc.tensor.matmul(
                    xt4ps[:], SEL_sb[:, 0:128], xt_sb[:], start=True, stop=True
                )
                xt4_sb = xt4pool.tile([128, TILE_ROWS], f32r, tag="xt4")
                nc.scalar.copy(xt4_sb[:], xt4ps[:])

                # xx16 into c4 rows 0:32
                nc.vector.tensor_mul(c4_sb[0:32, :], xt4_sb[0:32, :], rotps4[:])

                # pair-product chunks 0..3 (bf16):
                #   chunk_c[p] = x_{p%32} * x_{(p%32 + 4c + p//32)%32}
                chunk_tiles = []
                for c in range(4):
                    rotps = ps_rot.tile([128, TILE_ROWS], f32, tag="rot")
                    nc.tensor.matmul(
                        rotps[:],
                        SEL_sb[:, 128 * (c + 1) : 128 * (c + 2)],
                        xt_sb[:],
                        start=True,
                        stop=True,
                    )
                    xx_c = xxpool.tile([128, TILE_ROWS], bf16, tag=f"xx{c}")
                    nc.vector.tensor_mul(xx_c[:], xt4_sb[:], rotps[:])
                    chunk_tiles.append(xx_c)

                # main accumulating matmuls; the four bf16 chunks first,
                # c4 (f32r, slowest to produce) closes each group.
                for half in range(2):
                    psmain = ps_main.tile([128, 2 * M], f32, tag="main")
                    for s2 in range(2):
                        sub = half * 2 + s2
                        for c in range(4):
                            nc.tensor.matmul(
                                psmain[:, s2 * M : (s2 + 1) * M],
                                chunk_tiles[c][:, sub * 128 : (sub + 1) * 128],
                                U_sb[:, c, :],
                                start=(c == 0),
                                stop=False,
                            )
                        nc.tensor.matmul(
                            psmain[:, s2 * M : (s2 + 1) * M],
                            c4_sb[:, sub * 128 : (sub + 1) * 128],
                            U4_sb[:],
                            start=False,
                            stop=True,
                        )
                    for s2 in range(2):
                        sub = half * 2 + s2
                        expsc = exppool.tile([128, M], bf16, tag="exp")
                        col = t * 4 + sub
                        nc.scalar.activation(
                            expsc[:],
                            psmain[:, s2 * M : (s2 + 1) * M],
                            AF.Exp,
                            accum_out=sums_sb[:, col : col + 1],
                        )

            # epilogue: ll^T = Ln(sums); PE transpose; copy; DMA out
            llT = finpool.tile([128, NGROUPS], f32r)
            nc.scalar.activation(llT[:], sums_sb[:], AF.Ln)
            llps = ps_xt.tile([128, 128], f32r, tag="xtps")
            nc.tensor.transpose(llps[:], llT[:], EYER_sb[:])
            ll_sb = finpool.tile([128, 128], f32)
            nc.scalar.copy(ll_sb[:], llps[:])
            nc.sync.dma_start(OUT_d.rearrange("(c p) -> c p", c=128), ll_sb[:])

    nc.compile()
    return nc


def _host_prep(center, cov_inv_sqrt, weight, threshold):
    import ml_dtypes

    L = np.asarray(cov_inv_sqrt, dtype=np.float64)
    w = np.abs(np.asarray(weight, dtype=np.float64))
    pr = w / w.sum()
    A = np.einsum("mij,mkj->mik", L, L)
    sign, logdet = np.linalg.slogdet(A)
    logcoef = np.log(pr) + 0.5 * logdet
    c64 = np.asarray(center, dtype=np.float64)
    Ac = np.einsum("mkl,ml->mk", A, c64)
    term3 = np.einsum("mk,mk->m", c64, Ac)
    bias = logcoef - 0.5 * term3 - float(np.asarray(threshold).reshape(-1)[0])

    p = np.arange(128)
    U = np.zeros((128, 4, M), np.float32)
    for c in range(4):
        k = 4 * c + p // 32
        d1 = p % 32
        d2 = (d1 + k) % 32
        mult = np.where(k == 0, 1.0, 2.0)
        U[:, c, :] = (-0.5 * mult[:, None] * A[:, d1, d2].T).astype(np.float32)

    p32 = np.arange(32)
    U4 = np.zeros((128, M), np.float32)
    U4[0:32] = (-0.5 * A[:, p32, (p32 + 16) % 32].T).astype(np.float32)
    U4[32:64] = Ac.T.astype(np.float32)
    U4[64] = bias.astype(np.float32)

    SEL = np.zeros((128, 704), np.float32)
    dd = np.arange(128)
    SEL[:, 0:128] = (dd[:, None] == (p[None, :] % 32)).astype(np.float32)
    for c in range(4):
        k = 4 * c + p // 32
        b = (p % 32 + k) % 32
        SEL[:, 128 * (c + 1) : 128 * (c + 2)] = (dd[:, None] == b[None, :]).astype(
            np.float32
        )
    b16 = (p32 + 16) % 32
    SEL[:, 640:672] = (dd[:, None] == b16[None, :]).astype(np.float32)

    EYER = np.eye(128, dtype=np.float32)
    PAD4 = np.zeros((64, TILE_ROWS), np.float32)
    PAD4[0, :] = 1.0
    PADX = np.zeros((96, TILE_ROWS), np.float32)
    return U.astype(ml_dtypes.bfloat16), U4, SEL, EYER, PAD4, PADX


def kernel(X, center, cov_inv_sqrt, weight, threshold):
    global _PROGRAM
    from concourse.bass_utils import run_bass_kernel_spmd

    X = np.ascontiguousarray(np.asarray(X, dtype=np.float32))
    U, U4, SEL, EYER, PAD4, PADX = _host_prep(center, cov_inv_sqrt, weight, threshold)

    if _PROGRAM is None:
        _PROGRAM = _build_program()
    nc = _PROGRAM

    in_maps = []
    for k in range(NCORES):
        in_maps.append(
            {
                "X": X[k * NC_ROWS : (k + 1) * NC_ROWS],
                "U": U,
                "U4": U4,
                "SEL": SEL,
                "EYER": EYER,
                "PAD4": PAD4,
                "PADX": PADX,
            }
        )
    res = run_bass_kernel_spmd(nc, in_maps, list(range(NCORES)))
    out = np.concatenate([res.results[k]["out"] for k in range(NCORES)])
    return out.astype(np.float32)
